# revision 1
# baseline (speedup 1.0000x reference)
"""Trainium2 Bass kernel for nn_AimTransformer (B=8,T=32,D=384,NH=6,DEPTH=6).

Sharding: pure data-parallel over batch. Each of the 8 NeuronCores runs one
batch element end-to-end (conv frame-encoder -> temporal conv -> 6-layer
transformer -> coord head). Host folds BN into conv weights, builds the conv1
im2col, transposes weights, and precomputes the additive attention mask.

Device layout: feature-major activations [128 partitions x 3 chunks of D=384,
token cols]; 512 patch tokens (t-major: col = t*16+p) + cls at col 512.
Matmul inputs bf16, accumulation fp32 in PSUM.
"""

import numpy as np
import ml_dtypes

BF16 = ml_dtypes.bfloat16

B, T, HW = 8, 32, 128
D, NH, DEPTH, DFF = 384, 6, 6, 1536
HD = D // NH  # 64
P = 16        # patches per frame
S = T * P + 1  # 513 (cls at col 512 in our layout; reference has cls first)
EPS = 1e-5
NEG = -1e30

_PROG = None  # cached (nc, input_names)


# ---------------------------------------------------------------- host prep
def _fold_bn(w, g, b, m, v):
    inv = (g / np.sqrt(v + EPS)).astype(np.float32)
    wf = w * inv[:, None, None, None]
    bf = (b - m * inv).astype(np.float32)
    return wf.astype(np.float32), bf


def _im2col_conv1(frames_core):
    # frames_core [T,128,128] -> groups [8, 128, 4096]: partition 32*j+tap holds
    # frame (4g+j), tap=(kh*5+kw); col = y*64+x ; in[2y+kh-2, 2x+kw-2]
    fpad = np.zeros((T, HW + 4, HW + 4), np.float32)
    fpad[:, 2:-2, 2:-2] = frames_core
    taps = np.empty((T, 25, 4096), np.float32)
    for kh in range(5):
        for kw in range(5):
            taps[:, kh * 5 + kw, :] = fpad[:, kh:kh + 128:2, kw:kw + 128:2].reshape(T, -1)
    out = np.zeros((8, 4, 32, 4096), np.float32)
    out[:, :, :25, :] = taps.reshape(8, 4, 25, 4096)
    return out.reshape(8, 128, 4096)


def _prep_shared(inp):
    """Everything identical across cores, already in device layout."""
    d = {}
    w1, b1 = _fold_bn(inp['conv1_w'], inp['bn1_g'], inp['bn1_b'], inp['bn1_m'], inp['bn1_v'])
    w2, b2 = _fold_bn(inp['conv2_w'], inp['bn2_g'], inp['bn2_b'], inp['bn2_m'], inp['bn2_v'])
    w3, b3 = _fold_bn(inp['conv3_w'], inp['bn3_g'], inp['bn3_b'], inp['bn3_m'], inp['bn3_v'])
    # conv1: lhsT [K=25 tap, M=32 oc], replicated on 4 partition strips
    w1l = w1.reshape(32, 25).T.astype(np.float32)           # [25,32]
    w1r = np.zeros((128, 32), np.float32)
    for j in range(4):
        w1r[32 * j:32 * j + 25] = w1l
    d['w1r'] = w1r.astype(BF16)
    d['b1r'] = np.tile(b1[:, None], (4, 1)).astype(np.float32)  # [128,1]
    # conv2: lhsT per tap [K=32 ci, M=64 oc], replicated 4 strips -> [128,9,64]
    w2l = w2.transpose(2, 3, 1, 0).reshape(9, 32, 64)
    w2r = np.zeros((128, 9, 64), np.float32)
    for j in range(4):
        w2r[32 * j:32 * j + 32] = w2l.transpose(1, 0, 2)
    d['w2r'] = w2r.astype(BF16)
    d['b2'] = np.tile(b2, 2)[:, None].astype(np.float32)     # [128,1] (2 frames)
    # conv3: lhsT per tap [K=64 ci, M=128 oc] -> [64,9,128]
    w3l = w3.transpose(2, 3, 1, 0).reshape(9, 64, 128).transpose(1, 0, 2)  # [64,9,128]
    d['w3l'] = np.ascontiguousarray(np.concatenate([w3l, w3l], 0)).astype(BF16)  # [128,9,128]
    d['b3'] = b3[:, None].astype(np.float32)                 # [128,1]
    # proj (1x1): lhsT [K=128 ci, M=384], fold 1/16 pooling mean
    d['projT'] = (inp['proj_w'][:, :, 0, 0].T / 16.0).astype(BF16)  # [128,384]
    # temporal conv
    dww = np.asarray(inp['dw_w'])                            # [2,384,1,3]
    # dww[p, i, tap, chunk] = dw_w[i, chunk*128+p, 0, tap]
    d['dww'] = np.ascontiguousarray(
        dww[:, :, 0, :].transpose(1, 0, 2).reshape(3, 128, 2, 3).transpose(1, 2, 3, 0)
    ).astype(np.float32)
    dwb = np.asarray(inp['dw_b'])                            # [2,384]
    d['dwb'] = np.ascontiguousarray(
        dwb.T.reshape(3, 128, 2).transpose(1, 2, 0)).astype(np.float32)  # [128,2,3]
    pw = np.asarray(inp['pw_w'])                             # [2,384,384]
    pwT = np.zeros((128, 2, 3, 384), np.float32)
    for i in range(2):
        for kc in range(3):
            pwT[:, i, kc, :] = pw[i].T[kc * 128:(kc + 1) * 128, :]
    d['pwT'] = pwT.astype(BF16)
    d['pwb'] = np.asarray(inp['pw_b']).reshape(2, 3, 128)[None].astype(BF16)  # [1,2,3,128]
    d['tng'] = np.asarray(inp['tnorm_g']).reshape(3, 128).T.astype(np.float32)  # [128,3]
    d['tnb'] = np.asarray(inp['tnorm_b']).reshape(3, 128).T.astype(np.float32)
    # positional (cols 0:512) + cls init (col 512)
    fp = np.asarray(inp['frame_pos'])[:T, 0, :]              # [32,384]
    pp = np.asarray(inp['patch_pos'])[0]                     # [16,384]
    pos = (fp[:, None, :] + pp[None, :, :]).reshape(512, D)  # [512,384]
    clsv = (np.asarray(inp['cls_token']) + np.asarray(inp['cls_pos'])).reshape(D)
    posc = np.concatenate([pos, clsv[None]], 0).T            # [384,513]
    d['posc'] = posc.reshape(3, 128, S).transpose(1, 0, 2).astype(BF16)  # [128,3,513]
    # additive mask, transposed: maskT[k,q]; frame(tok)=tok//16; reference is
    # "reverse causal": patch q attends patch k iff frame(k) >= frame(q);
    # patch q never attends cls; cls attends everything.
    ids = np.repeat(np.arange(T), P)
    mT = np.zeros((S, S), np.float32)
    mT[:512, :512] = np.where(ids[:, None] >= ids[None, :], 0.0, NEG)  # [k,q]
    mT[512, :512] = NEG
    mT[:, 512] = 0.0
    mpad = np.zeros((640, S), np.float32)
    mpad[:S] = mT
    d['maskT'] = mpad.reshape(5, 128, S).transpose(1, 0, 2).astype(np.float32)  # [128,5,513]
    # transformer weights
    scale = 1.0 / np.sqrt(HD)
    qkT = np.zeros((DEPTH, 128, 3, 768), np.float32)
    vwT = np.zeros((DEPTH, 128, 3, 384), np.float32)
    ouT = np.zeros((DEPTH, 128, 3, 384), np.float32)
    f1T = np.zeros((DEPTH, 128, 3, 1536), np.float32)
    f2T = np.zeros((DEPTH, 128, 12, 384), np.float32)
    for i in range(DEPTH):
        w = np.asarray(inp['qkv_w'][i])                      # [1152,384]
        wq = (w[:384] * scale)
        wk = w[384:768]
        wv = w[768:]
        qk = np.concatenate([wq, wk], 0).T                   # [384,768]
        for kc in range(3):
            qkT[i, :, kc, :] = qk[kc * 128:(kc + 1) * 128]
            vwT[i, :, kc, :] = wv.T[kc * 128:(kc + 1) * 128]
            ouT[i, :, kc, :] = np.asarray(inp['out_w'][i]).T[kc * 128:(kc + 1) * 128]
            f1T[i, :, kc, :] = np.asarray(inp['ffn_w1'][i]).T[kc * 128:(kc + 1) * 128]
        for kc in range(12):
            f2T[i, :, kc, :] = np.asarray(inp['ffn_w2'][i]).T[kc * 128:(kc + 1) * 128]
    d['qkT'] = qkT.astype(BF16)
    d['vwT'] = vwT.astype(BF16)
    d['ouT'] = ouT.astype(BF16)
    d['f1T'] = f1T.astype(BF16)
    d['f2T'] = f2T.astype(BF16)
    qb = np.asarray(inp['qkv_b'])                            # [6,1152]
    qbs = qb.copy()
    qbs[:, :384] *= scale
    d['qkb'] = qbs[:, :768].reshape(DEPTH, 1, 6, 128).astype(BF16)
    d['vb'] = qbs[:, 768:].reshape(DEPTH, 1, 384).astype(BF16)
    d['outb'] = np.asarray(inp['out_b']).reshape(DEPTH, 1, 3, 128).astype(BF16)
    d['f1b'] = np.asarray(inp['ffn_b1']).reshape(DEPTH, 12, 128).transpose(0, 2, 1).astype(np.float32)  # [DEPTH,128,12]
    d['f2b'] = np.asarray(inp['ffn_b2']).reshape(DEPTH, 1, 3, 128).astype(BF16)
    for nm, key in (('ln1g', 'ln1_g'), ('ln1b', 'ln1_b'), ('ln2g', 'ln2_g'), ('ln2b', 'ln2_b')):
        d[nm] = np.asarray(inp[key]).reshape(DEPTH, 3, 128).transpose(0, 2, 1).astype(np.float32)  # [DEPTH,128,3]
    d['hlng'] = np.asarray(inp['head_ln_g']).reshape(3, 128).T.astype(np.float32)
    d['hlnb'] = np.asarray(inp['head_ln_b']).reshape(3, 128).T.astype(np.float32)
    hw1 = np.asarray(inp['head_w1']).T                       # [384,384]
    d['hw1T'] = np.ascontiguousarray(hw1.reshape(3, 128, 384).transpose(1, 0, 2)).astype(BF16)  # [128,3,384]
    d['hb1'] = np.asarray(inp['head_b1']).reshape(3, 128).T.astype(np.float32)  # [128,3]
    d['hw2T'] = np.ascontiguousarray(np.asarray(inp['head_w2']).T.reshape(3, 128, 2).transpose(1, 0, 2)).astype(BF16)
    d['hb2'] = np.asarray(inp['head_b2']).reshape(2, 1).astype(np.float32)
    return d


# ------------------------------------------------------------- device build
def _build_program():
    import concourse.mybir as mybir
    import concourse.tile as tile
    from concourse import bacc

    MM = mybir.dt.bfloat16
    F32 = mybir.dt.float32
    AF = mybir.ActivationFunctionType
    OP = mybir.AluOpType

    nc = bacc.Bacc("TRN2", target_bir_lowering=False, debug=False,
                   enable_asserts=True, num_devices=8)

    def par(name, shape, dt=MM):
        return nc.declare_dram_parameter(name, list(shape), dt, isOutput=False)

    dm1 = par('m1', [8, 128, 4096])
    dw1r = par('w1r', [128, 32]); db1r = par('b1r', [128, 1], F32)
    dw2r = par('w2r', [128, 9, 64]); db2 = par('b2', [128, 1], F32)
    dw3l = par('w3l', [128, 9, 128]); db3 = par('b3', [128, 1], F32)
    dprojT = par('projT', [128, 384])
    ddww = par('dww', [128, 2, 3, 3], F32)
    ddwb = par('dwb', [128, 2, 3], F32)
    dpwT = par('pwT', [128, 2, 3, 384])
    dpwb = par('pwb', [1, 2, 3, 128])
    dtng = par('tng', [128, 3], F32); dtnb = par('tnb', [128, 3], F32)
    dposc = par('posc', [128, 3, S])
    dmaskT = par('maskT', [128, 5, S], F32)
    dqkT = par('qkT', [DEPTH, 128, 3, 768])
    dvwT = par('vwT', [DEPTH, 128, 3, 384])
    douT = par('ouT', [DEPTH, 128, 3, 384])
    df1T = par('f1T', [DEPTH, 128, 3, 1536])
    df2T = par('f2T', [DEPTH, 128, 12, 384])
    dqkb = par('qkb', [DEPTH, 1, 6, 128])
    dvb = par('vb', [DEPTH, 1, 384])
    doutb = par('outb', [DEPTH, 1, 3, 128])
    df1b = par('f1b', [DEPTH, 128, 12], F32)
    df2b = par('f2b', [DEPTH, 1, 3, 128])
    dln = {nm: par(nm, [DEPTH, 128, 3], F32) for nm in ('ln1g', 'ln1b', 'ln2g', 'ln2b')}
    dhlng = par('hlng', [128, 3], F32); dhlnb = par('hlnb', [128, 3], F32)
    dhw1T = par('hw1T', [128, 3, 384]); dhb1 = par('hb1', [128, 3], F32)
    dhw2T = par('hw2T', [128, 3, 2]); dhb2 = par('hb2', [2, 1], F32)
    dout = nc.declare_dram_parameter('out', [2, 1], F32, isOutput=True)

    with tile.TileContext(nc) as tc:
        _emit(nc, tc, tile, mybir, MM, F32, AF, OP, locals())
    return nc


def _emit(nc, tc, tile, mybir, MM, F32, AF, OP, dr):
    def CC(n):  # col chunks of n
        return [(0, 512), (512, 1)] if n == 513 else [(0, n)]

    with tc.tile_pool(name="const", bufs=1) as cp:
        ones_col = cp.tile([128, 1], MM, tag="ones_col")
        nc.gpsimd.memset(ones_col[:], 1.0)
        ones_row = cp.tile([1, 512], MM, tag="ones_row")
        nc.gpsimd.memset(ones_row[:], 1.0)
        ones_k1 = cp.tile([1, 128], MM, tag="ones_k1")
        nc.gpsimd.memset(ones_k1[:], 1.0)
        epst = cp.tile([1, 1], F32, tag="epst")
        nc.gpsimd.memset(epst[:], EPS)

        maskT = cp.tile([128, 5, S], F32, tag="maskT")
        nc.sync.dma_start(maskT[:], dr['dmaskT'][:])
        posc = cp.tile([128, 3, S], MM, tag="posc")
        nc.sync.dma_start(posc[:], dr['dposc'][:])

        X0 = cp.tile([128, 3, 512], MM, tag="X0")   # tokens after proj
        xpool = cp.tile([128, 32, 4, 4], MM, tag="xpool")

        # ---------------- conv stack ----------------
        with tc.tile_pool(name="convw", bufs=1) as cw, \
             tc.tile_pool(name="convb", bufs=3) as cb, \
             tc.tile_pool(name="convps", bufs=2, space="PSUM") as cps:
            w1r = cw.tile([128, 32], MM, tag="w1r"); nc.sync.dma_start(w1r[:], dr['dw1r'][:])
            b1r = cw.tile([128, 1], F32, tag="b1r"); nc.sync.dma_start(b1r[:], dr['db1r'][:])
            w2r = cw.tile([128, 9, 64], MM, tag="w2r"); nc.sync.dma_start(w2r[:], dr['dw2r'][:])
            b2 = cw.tile([128, 1], F32, tag="b2"); nc.sync.dma_start(b2[:], dr['db2'][:])
            w3l = cw.tile([128, 9, 128], MM, tag="w3l"); nc.sync.dma_start(w3l[:], dr['dw3l'][:])
            b3 = cw.tile([128, 1], F32, tag="b3"); nc.sync.dma_start(b3[:], dr['db3'][:])
            projT = cw.tile([128, 384], MM, tag="projT"); nc.sync.dma_start(projT[:], dr['dprojT'][:])

            for g in range(8):
                m1t = cb.tile([128, 4096], MM, tag="m1t")
                nc.sync.dma_start(m1t[:], dr['dm1'][g])
                # conv1: 4 frames packed on partition strips; quarters of 1024
                x1p = cb.tile([128, 66, 66], MM, tag="x1p")
                nc.vector.memset(x1p[:, 0::65, :], 0.0)   # top/bottom border rows
                nc.vector.memset(x1p[:, :, 0::65], 0.0)   # left/right border cols
                for q in range(4):
                    p1 = cps.tile([128, 1024], F32, tag="c1")
                    for j in range(4):
                        for c in range(2):
                            cc = q * 2 + c
                            nc.tensor.matmul(
                                p1[32 * j:32 * j + 32, c * 512:(c + 1) * 512],
                                w1r[32 * j:32 * j + 25, :],
                                m1t[32 * j:32 * j + 25, cc * 512:(cc + 1) * 512],
                                start=True, stop=True,
                                tile_position=(32 * j, 32 * j))
                    nc.scalar.activation(
                        x1p[:, 1 + 16 * q:1 + 16 * q + 16, 1:65],
                        p1[:].rearrange("p (c r x) -> p (c r) x", c=2, x=64),
                        AF.Gelu, bias=b1r[:])
                # conv2: frame pairs packed on psum col strips
                x2pa = cb.tile([128, 34, 34], MM, tag="x2pa")
                x2pb = cb.tile([128, 34, 34], MM, tag="x2pb")
                for pair, x2p in ((0, x2pa), (1, x2pb)):
                    nc.vector.memset(x2p[:, 0::33, :], 0.0)
                    nc.vector.memset(x2p[:, :, 0::33], 0.0)
                    for c2 in range(2):  # output row chunks of 16
                        p2 = cps.tile([128, 512], F32, tag="c2")
                        for jj in range(2):
                            j = 2 * pair + jj
                            for kh in range(3):
                                for kw in range(3):
                                    nc.tensor.matmul(
                                        p2[64 * jj:64 * jj + 64, :],
                                        w2r[32 * j:32 * j + 32, kh * 3 + kw, :],
                                        x1p[32 * j:32 * j + 32,
                                            kh + 32 * c2:kh + 32 * c2 + 32:2,
                                            kw:kw + 64:2],
                                        start=(kh == 0 and kw == 0),
                                        stop=(kh == 2 and kw == 2),
                                        tile_position=(32 * j, 64 * jj))
                        nc.scalar.activation(
                            x2p[:, 1 + 16 * c2:1 + 16 * c2 + 16, 1:33],
                            p2[:].rearrange("p (r x) -> p r x", x=32),
                            AF.Gelu, bias=b2[:])
                # conv3 + pool per frame
                for j in range(4):
                    pair, jj = j // 2, j % 2
                    x2p = x2pa if pair == 0 else x2pb
                    p3 = cps.tile([128, 256], F32, tag="c3")
                    for kh in range(3):
                        for kw in range(3):
                            nc.tensor.matmul(
                                p3[:],
                                w3l[64 * jj:64 * jj + 64, kh * 3 + kw, :],
                                x2p[64 * jj:64 * jj + 64,
                                    kh:kh + 32:2, kw:kw + 32:2],
                                start=(kh == 0 and kw == 0),
                                stop=(kh == 2 and kw == 2),
                                tile_position=(64 * jj, 0))
                    x3 = cb.tile([128, 16, 16], MM, tag="x3")
                    nc.scalar.activation(x3[:], p3[:].rearrange("p (r x) -> p r x", x=16),
                                         AF.Gelu, bias=b3[:])
                    # 4x4 mean pool (1/16 folded into projT): sum x then y
                    s1 = cb.tile([128, 16, 4], MM, tag="pools1")
                    nc.vector.tensor_add(s1[:], x3[:, :, 0::4], x3[:, :, 1::4])
                    s2 = cb.tile([128, 16, 4], MM, tag="pools2")
                    nc.vector.tensor_add(s2[:], x3[:, :, 2::4], x3[:, :, 3::4])
                    nc.vector.tensor_add(s1[:], s1[:], s2[:])
                    t4 = cb.tile([128, 4, 4], MM, tag="poolt4")
                    nc.vector.tensor_add(t4[:], s1[:, 0::4, :], s1[:, 1::4, :])
                    t5 = cb.tile([128, 4, 4], MM, tag="poolt5")
                    nc.vector.tensor_add(t5[:], s1[:, 2::4, :], s1[:, 3::4, :])
                    nc.vector.tensor_add(xpool[:, 4 * g + j], t4[:], t5[:])

            # proj -> tokens X0 [128,3,512]
            for oc in range(3):
                pp = cps.tile([128, 512], F32, tag="c2")
                nc.tensor.matmul(pp[:], projT[:, oc * 128:(oc + 1) * 128],
                                 xpool[:].rearrange("p a b c -> p (a b c)"),
                                 start=True, stop=True)
                nc.vector.tensor_copy(X0[:, oc, :], pp[:])

        # ---------------- post-conv pools ----------------
        with tc.tile_pool(name="acts", bufs=1) as ap, \
             tc.tile_pool(name="attp", bufs=2) as atp, \
             tc.tile_pool(name="wts", bufs=2) as wp, \
             tc.tile_pool(name="ps", bufs=2, space="PSUM") as ps, \
             tc.tile_pool(name="st", bufs=2, space="PSUM") as st:

            # ---------------- temporal conv block ----------------
            xp = ap.tile([128, 3, 576], MM, tag="xp")
            nc.vector.memset(xp[:, :, 0:32], 0.0)
            nc.vector.memset(xp[:, :, 544:576], 0.0)
            nc.vector.tensor_copy(xp[:, :, 32:544], X0[:])
            htile = ap.tile([128, 3, 512], MM, tag="htile")
            tmp = ap.tile([128, 3, 512], MM, tag="tctmp")
            dww = ap.tile([128, 2, 3, 3], F32, tag="dww"); nc.sync.dma_start(dww[:], dr['ddww'][:])
            dwb = ap.tile([128, 2, 3], F32, tag="dwb"); nc.sync.dma_start(dwb[:], dr['ddwb'][:])
            pwT = ap.tile([128, 2, 3, 384], MM, tag="pwT"); nc.sync.dma_start(pwT[:], dr['dpwT'][:])
            pwb = ap.tile([1, 2, 3, 128], MM, tag="pwb"); nc.sync.dma_start(pwb[:], dr['dpwb'][:])
            for i in range(2):
                sh = 16 * (i + 1)  # dilation 1,2 -> col shift 16,32
                for c in range(3):
                    nc.vector.tensor_scalar(
                        out=htile[:, c], in0=xp[:, c, 32 - sh:544 - sh],
                        scalar1=dww[:, i, 0, c][:, None], scalar2=None, op0=OP.mult)
                    nc.vector.tensor_scalar(
                        out=tmp[:, c], in0=xp[:, c, 32:544],
                        scalar1=dww[:, i, 1, c][:, None], scalar2=None, op0=OP.mult)
                    nc.vector.tensor_add(htile[:, c], htile[:, c], tmp[:, c])
                    nc.vector.tensor_scalar(
                        out=tmp[:, c], in0=xp[:, c, 32 + sh:544 + sh],
                        scalar1=dww[:, i, 2, c][:, None], scalar2=None, op0=OP.mult)
                    nc.vector.tensor_add(htile[:, c], htile[:, c], tmp[:, c])
                    nc.scalar.activation(htile[:, c], htile[:, c], AF.Gelu,
                                         bias=dwb[:, i, c][:, None])
                for oc in range(3):
                    pw_ps = ps.tile([128, S], F32, tag="ps")
                    for kc in range(3):
                        nc.tensor.matmul(pw_ps[:, 0:512],
                                         pwT[:, i, kc, oc * 128:(oc + 1) * 128],
                                         htile[:, kc], start=(kc == 0), stop=False)
                    nc.tensor.matmul(pw_ps[:, 0:512], pwb[:, i, oc, :], ones_row[:],
                                     start=False, stop=True)
                    nc.vector.tensor_add(xp[:, oc, 32:544], xp[:, oc, 32:544],
                                         pw_ps[:, 0:512])

            lnin = ap.tile([128, 3, S], MM, tag="lnin")
            nc.vector.tensor_add(lnin[:, :, 0:512], xp[:, :, 32:544], X0[:])
            tng = ap.tile([128, 3], F32, tag="tng"); nc.sync.dma_start(tng[:], dr['dtng'][:])
            tnb = ap.tile([128, 3], F32, tag="tnb"); nc.sync.dma_start(tnb[:], dr['dtnb'][:])

            x = ap.tile([128, 3, S], MM, tag="x")
            sq = ap.tile([128, 3, S], MM, tag="sq")
            stat_mu = ap.tile([1, S], MM, tag="stat_mu")
            stat_iv = ap.tile([1, S], MM, tag="stat_iv")
            musq = ap.tile([1, S], MM, tag="musq")
            bc_mu = ap.tile([128, S], MM, tag="bc_mu")
            bc_iv = ap.tile([128, S], MM, tag="bc_iv")
            lntmp = ap.tile([128, 3, S], MM, tag="lntmp")

            def layer_norm(src, ncols, g_ap, b_ap, dst, dst_off=0):
                """src [128,3,ncols] -> dst[:, :, off:off+ncols] = LN over d."""
                nc.scalar.activation(sq[:, :, :ncols], src, AF.Square)
                pss = st.tile([1, S], F32, tag="st")
                psq = st.tile([1, S], F32, tag="st")
                for (c0, cw) in CC(ncols):
                    for kc in range(3):
                        nc.tensor.matmul(pss[:, c0:c0 + cw], ones_col[:],
                                         src[:, kc, c0:c0 + cw],
                                         start=(kc == 0), stop=(kc == 2))
                        nc.tensor.matmul(psq[:, c0:c0 + cw], ones_col[:],
                                         sq[:, kc, c0:c0 + cw],
                                         start=(kc == 0), stop=(kc == 2))
                nc.vector.tensor_scalar(out=stat_mu[:, :ncols], in0=pss[:, :ncols],
                                        scalar1=1.0 / D, scalar2=None, op0=OP.mult)
                nc.vector.tensor_mul(musq[:, :ncols], stat_mu[:, :ncols], stat_mu[:, :ncols])
                nc.vector.tensor_scalar(out=stat_iv[:, :ncols], in0=psq[:, :ncols],
                                        scalar1=1.0 / D, scalar2=None, op0=OP.mult)
                nc.vector.tensor_sub(stat_iv[:, :ncols], stat_iv[:, :ncols], musq[:, :ncols])
                nc.scalar.activation(stat_iv[:, :ncols], stat_iv[:, :ncols], AF.Sqrt, bias=epst[:])
                with nc.allow_low_precision(reason="bf16 LN inv-std, matches bf16 activations"):
                    nc.vector.reciprocal(stat_iv[:, :ncols], stat_iv[:, :ncols])
                nc.vector.tensor_scalar(out=stat_mu[:, :ncols], in0=stat_mu[:, :ncols],
                                        scalar1=-1.0, scalar2=None, op0=OP.mult)
                psbm = ps.tile([128, S], F32, tag="ps")
                psbi = ps.tile([128, S], F32, tag="ps")
                for (c0, cw) in CC(ncols):
                    nc.tensor.matmul(psbm[:, c0:c0 + cw], ones_k1[:], stat_mu[:, c0:c0 + cw],
                                     start=True, stop=True)
                    nc.tensor.matmul(psbi[:, c0:c0 + cw], ones_k1[:], stat_iv[:, c0:c0 + cw],
                                     start=True, stop=True)
                nc.vector.tensor_copy(bc_mu[:, :ncols], psbm[:, :ncols])
                nc.vector.tensor_copy(bc_iv[:, :ncols], psbi[:, :ncols])
                for c in range(3):
                    nc.vector.tensor_add(lntmp[:, c, :ncols], src[:, c, :], bc_mu[:, :ncols])
                    nc.vector.tensor_mul(lntmp[:, c, :ncols], lntmp[:, c, :ncols], bc_iv[:, :ncols])
                    nc.vector.tensor_scalar(
                        out=dst[:, c, dst_off:dst_off + ncols], in0=lntmp[:, c, :ncols],
                        scalar1=g_ap[:, c][:, None], scalar2=b_ap[:, c][:, None],
                        op0=OP.mult, op1=OP.add)

            layer_norm(lnin[:, :, 0:512], 512, tng, tnb, x)
            nc.vector.tensor_add(x[:, :, 0:512], x[:, :, 0:512], posc[:, :, 0:512])
            nc.vector.tensor_copy(x[:, :, 512:513], posc[:, :, 512:513])

            # ---------------- transformer ----------------
            h = ap.tile([128, 3, S], MM, tag="h")
            qk = ap.tile([128, 6, S], MM, tag="qk")
            VT = ap.tile([128, 5, 384], MM, tag="VT")
            attno = ap.tile([128, 3, S], MM, tag="attno")
            f1 = ap.tile([128, 12, S], MM, tag="f1")
            rsb = ap.tile([1, S], MM, tag="rsb")
            rbc = ap.tile([64, S], MM, tag="rbc")

            for li in range(DEPTH):
                qkT = wp.tile([128, 3, 768], MM, tag="qkT"); nc.sync.dma_start(qkT[:], dr['dqkT'][li])
                vwT = wp.tile([128, 3, 384], MM, tag="vwT"); nc.sync.dma_start(vwT[:], dr['dvwT'][li])
                ouT = wp.tile([128, 3, 384], MM, tag="ouT"); nc.sync.dma_start(ouT[:], dr['douT'][li])
                f1T = wp.tile([128, 3, 1536], MM, tag="f1T"); nc.sync.dma_start(f1T[:], dr['df1T'][li])
                f2T = wp.tile([128, 12, 384], MM, tag="f2T"); nc.sync.dma_start(f2T[:], dr['df2T'][li])
                qkb = wp.tile([1, 6, 128], MM, tag="qkb"); nc.sync.dma_start(qkb[:], dr['dqkb'][li])
                vb = wp.tile([1, 384], MM, tag="vb"); nc.sync.dma_start(vb[:], dr['dvb'][li])
                outb = wp.tile([1, 3, 128], MM, tag="outb"); nc.sync.dma_start(outb[:], dr['doutb'][li])
                f1b = wp.tile([128, 12], F32, tag="f1b"); nc.sync.dma_start(f1b[:], dr['df1b'][li])
                f2b = wp.tile([1, 3, 128], MM, tag="f2b"); nc.sync.dma_start(f2b[:], dr['df2b'][li])
                ln1g = wp.tile([128, 3], F32, tag="ln1g"); nc.sync.dma_start(ln1g[:], dr['dln']['ln1g'][li])
                ln1b = wp.tile([128, 3], F32, tag="ln1b"); nc.sync.dma_start(ln1b[:], dr['dln']['ln1b'][li])
                ln2g = wp.tile([128, 3], F32, tag="ln2g"); nc.sync.dma_start(ln2g[:], dr['dln']['ln2g'][li])
                ln2b = wp.tile([128, 3], F32, tag="ln2b"); nc.sync.dma_start(ln2b[:], dr['dln']['ln2b'][li])

                layer_norm(x[:, :, :], S, ln1g, ln1b, h)

                for oc in range(6):  # q,k projections (q pre-scaled on host)
                    pm = ps.tile([128, S], F32, tag="ps")
                    for (c0, cw) in CC(S):
                        for kc in range(3):
                            nc.tensor.matmul(pm[:, c0:c0 + cw],
                                             qkT[:, kc, oc * 128:(oc + 1) * 128],
                                             h[:, kc, c0:c0 + cw],
                                             start=(kc == 0), stop=False)
                        nc.tensor.matmul(pm[:, c0:c0 + cw], qkb[:, oc, :],
                                         ones_row[:, :cw], start=False, stop=True)
                    nc.vector.tensor_copy(qk[:, oc, :], pm[:])
                for tt in range(5):  # V transposed [tok, vd]
                    tw = 128 if tt < 4 else 1
                    pv = ps.tile([128, S], F32, tag="ps")
                    for kc in range(3):
                        nc.tensor.matmul(pv[:tw, 0:384], h[:, kc, tt * 128:tt * 128 + tw],
                                         vwT[:, kc, :], start=(kc == 0), stop=False)
                    nc.tensor.matmul(pv[:tw, 0:384], ones_row[:, :tw], vb[:],
                                     start=False, stop=True)
                    nc.vector.tensor_copy(VT[:tw, tt, :], pv[:tw, 0:384])
                for hh in range(6):
                    po = (hh % 2) * 64
                    chq = hh // 2
                    chk = 3 + hh // 2
                    AT = atp.tile([128, 5, S], MM, tag="AT")
                    for kt in range(5):
                        kw_ = 128 if kt < 4 else 1
                        psc = ps.tile([128, S], F32, tag="ps")
                        for (c0, cw) in CC(S):
                            nc.tensor.matmul(psc[:kw_, c0:c0 + cw],
                                             qk[po:po + 64, chk, kt * 128:kt * 128 + kw_],
                                             qk[po:po + 64, chq, c0:c0 + cw],
                                             start=True, stop=True)
                        nc.vector.tensor_add(AT[:kw_, kt, :], psc[:kw_, :], maskT[:kw_, kt, :])
                        nc.scalar.activation(AT[:kw_, kt, :], AT[:kw_, kt, :], AF.Exp)
                    prs = st.tile([1, S], F32, tag="st")
                    for (c0, cw) in CC(S):
                        for kt in range(5):
                            kw_ = 128 if kt < 4 else 1
                            nc.tensor.matmul(prs[:, c0:c0 + cw], ones_col[:kw_, :],
                                             AT[:kw_, kt, c0:c0 + cw],
                                             start=(kt == 0), stop=(kt == 4))
                    with nc.allow_low_precision(reason="bf16 softmax denom, matches bf16 activations"):
                        nc.vector.reciprocal(rsb[:], prs[:])
                    pbc = ps.tile([64, S], F32, tag="ps")
                    for (c0, cw) in CC(S):
                        nc.tensor.matmul(pbc[:, c0:c0 + cw], ones_k1[:, :64],
                                         rsb[:, c0:c0 + cw], start=True, stop=True)
                    nc.vector.tensor_copy(rbc[:], pbc[:])
                    pav = ps.tile([64, S], F32, tag="ps")
                    for (c0, cw) in CC(S):
                        for kt in range(5):
                            kw_ = 128 if kt < 4 else 1
                            nc.tensor.matmul(pav[:, c0:c0 + cw],
                                             VT[:kw_, kt, hh * 64:hh * 64 + 64],
                                             AT[:kw_, kt, c0:c0 + cw],
                                             start=(kt == 0), stop=(kt == 4))
                    nc.vector.tensor_mul(attno[po:po + 64, chq, :], pav[:], rbc[:])
                for oc in range(3):  # out proj + residual
                    pm = ps.tile([128, S], F32, tag="ps")
                    for (c0, cw) in CC(S):
                        for kc in range(3):
                            nc.tensor.matmul(pm[:, c0:c0 + cw],
                                             ouT[:, kc, oc * 128:(oc + 1) * 128],
                                             attno[:, kc, c0:c0 + cw],
                                             start=(kc == 0), stop=False)
                        nc.tensor.matmul(pm[:, c0:c0 + cw], outb[:, oc, :],
                                         ones_row[:, :cw], start=False, stop=True)
                    nc.vector.tensor_add(x[:, oc, :], x[:, oc, :], pm[:])
                layer_norm(x[:, :, :], S, ln2g, ln2b, h)
                for oc in range(12):
                    pm = ps.tile([128, S], F32, tag="ps")
                    for (c0, cw) in CC(S):
                        for kc in range(3):
                            nc.tensor.matmul(pm[:, c0:c0 + cw],
                                             f1T[:, kc, oc * 128:(oc + 1) * 128],
                                             h[:, kc, c0:c0 + cw],
                                             start=(kc == 0), stop=(kc == 2))
                    nc.scalar.activation(f1[:, oc, :], pm[:], AF.Gelu,
                                         bias=f1b[:, oc][:, None])
                for oc in range(3):
                    pm = ps.tile([128, S], F32, tag="ps")
                    for (c0, cw) in CC(S):
                        for kc in range(12):
                            nc.tensor.matmul(pm[:, c0:c0 + cw],
                                             f2T[:, kc, oc * 128:(oc + 1) * 128],
                                             f1[:, kc, c0:c0 + cw],
                                             start=(kc == 0), stop=False)
                        nc.tensor.matmul(pm[:, c0:c0 + cw], f2b[:, oc, :],
                                         ones_row[:, :cw], start=False, stop=True)
                    nc.vector.tensor_add(x[:, oc, :], x[:, oc, :], pm[:])

            # ---------------- head ----------------
            hlng = ap.tile([128, 3], F32, tag="hlng"); nc.sync.dma_start(hlng[:], dr['dhlng'][:])
            hlnb = ap.tile([128, 3], F32, tag="hlnb"); nc.sync.dma_start(hlnb[:], dr['dhlnb'][:])
            hw1T = ap.tile([128, 3, 384], MM, tag="hw1T"); nc.sync.dma_start(hw1T[:], dr['dhw1T'][:])
            hb1 = ap.tile([128, 3], F32, tag="hb1"); nc.sync.dma_start(hb1[:], dr['dhb1'][:])
            hw2T = ap.tile([128, 3, 2], MM, tag="hw2T"); nc.sync.dma_start(hw2T[:], dr['dhw2T'][:])
            hb2 = ap.tile([2, 1], F32, tag="hb2"); nc.sync.dma_start(hb2[:], dr['dhb2'][:])

            hcls = ap.tile([128, 3, 1], MM, tag="hcls")
            layer_norm(x[:, :, 512:513], 1, hlng, hlnb, hcls)
            h1 = ap.tile([128, 3, 1], MM, tag="h1")
            for oc in range(3):
                pm = ps.tile([128, S], F32, tag="ps")
                for kc in range(3):
                    nc.tensor.matmul(pm[:, 0:1], hw1T[:, kc, oc * 128:(oc + 1) * 128],
                                     hcls[:, kc, :], start=(kc == 0), stop=(kc == 2))
                nc.scalar.activation(h1[:, oc, :], pm[:, 0:1], AF.Gelu, bias=hb1[:, oc][:, None])
            pm2 = ps.tile([128, S], F32, tag="ps")
            for kc in range(3):
                nc.tensor.matmul(pm2[0:2, 0:1], hw2T[:, kc, :], h1[:, kc, :],
                                 start=(kc == 0), stop=(kc == 2))
            res = ap.tile([2, 1], F32, tag="res")
            nc.scalar.activation(res[:], pm2[0:2, 0:1], AF.Sigmoid, bias=hb2[:])
            nc.sync.dma_start(dr['dout'][:], res[:])


# ---------------------------------------------------------------- entry
def kernel(**inputs):
    global _PROG
    from concourse.bass_utils import run_bass_kernel_spmd

    inputs = {k: np.asarray(v) for k, v in inputs.items()}
    shared = _prep_shared(inputs)
    frames = np.asarray(inputs['frames'], np.float32)  # [8,32,1,128,128]

    if _PROG is None:
        nc = _build_program()
        nc.compile()
        nc.compile = lambda: None  # finalize() re-invokes compile; make it a no-op
        _PROG = nc
    nc = _PROG

    in_maps = []
    for core in range(8):
        m = dict(shared)
        m['m1'] = _im2col_conv1(frames[core, :, 0]).astype(BF16)
        in_maps.append(m)
    import os
    import time
    res = run_bass_kernel_spmd(nc, in_maps, list(range(8)))
    if os.environ.get('BASS_PROFILE'):
        # No NTFF hook in this container; report warm-run wall time
        # (includes input upload) as an upper bound on HW exec.
        t0 = time.time()
        res = run_bass_kernel_spmd(nc, in_maps, list(range(8)))
        t1 = time.time()
        print('HW exec time:', int((t1 - t0) * 1e9), 'ns  (warm wall, upper bound)')
    out = np.stack([res.results[c]['out'].reshape(2) for c in range(8)], 0)
    return out.astype(np.float32)

